# revision 1
# baseline (speedup 1.0000x reference)
"""Multi-head self-attention (RoPE + causal softmax) Bass kernel for TRN2.

Problem: B=2, H=16, S=2048, D_HEAD=64, fp32 I/O.
Sharding: 32 head-instances (B*H) split 4-per-core across 8 NeuronCores;
no cross-device communication.

Per-core kernel structure (4 heads, S=2048):
  - Q,K arrive host-pre-folded as head-pair tiles (128 partitions = s%128,
    free = [s_tile, headA_d | headB_d]).  RoPE is applied on DVE in this
    natural layout (pairing along the free dim), output in bf16.
  - XBAR DMA-transpose produces Q^T/K^T layouts (d on partitions, s on
    free), with two heads stacked on partitions 0-63 / 64-127.
  - Scores are computed transposed: S^T[k, q] = K^T.T @ Q^T per 128-row
    k-tile, causally trimmed to q >= k_tile_start, in 1024-column q-chunks.
  - exp(s/8) runs on ScalarE straight out of PSUM into bf16 SBUF (no-max
    softmax: scores are ~N(0,1) so exp never overflows).  Diagonal blocks
    get a 128x128 triangular mask via GPSIMD multiply.
  - V is shipped bf16 with a ones-column appended: out^T(65 x q) accumulates
    attn@[V|1] over k-tiles; row 64 is the softmax denominator.
  - 65x128 PE transposes + DVE reciprocal/scale produce the normalized
    (q, d) output tiles, DMA'd back to DRAM.
"""

import numpy as np
import ml_dtypes

import concourse.bass as bass
import concourse.tile as tile
from concourse import bacc, mybir
from concourse.bass_utils import run_bass_kernel_spmd

F32 = mybir.dt.float32
BF16 = mybir.dt.bfloat16
EXP = mybir.ActivationFunctionType.Exp

B, H, S_FULL, DH = 2, 16, 2048, 64
N_CORES = 8
HEADS_PER_CORE = (B * H) // N_CORES  # 4


# ---------------------------------------------------------------- device IR


def build_nc(n_heads=HEADS_PER_CORE, S=S_FULL, chunk=512, num_devices=N_CORES):
    """Build + compile the per-core Bass program (same program on all cores)."""
    NT = S // 128            # number of 128-row s-tiles
    npairs = n_heads // 2

    nc = bacc.Bacc(
        "TRN2", target_bir_lowering=False, debug=False, num_devices=num_devices
    )

    qp = nc.dram_tensor("qp", [npairs, 128, NT * 128], F32, kind="ExternalInput").ap()
    kp = nc.dram_tensor("kp", [npairs, 128, NT * 128], F32, kind="ExternalInput").ap()
    vx = nc.dram_tensor("vx", [n_heads, 128, NT * 65], BF16, kind="ExternalInput").ap()
    cosf = nc.dram_tensor("cosf", [128, NT * 128], BF16, kind="ExternalInput").ap()
    sinf = nc.dram_tensor("sinf", [128, NT * 128], BF16, kind="ExternalInput").ap()
    tri = nc.dram_tensor("tri", [128, 256], BF16, kind="ExternalInput").ap()
    ident = nc.dram_tensor("ident", [128, 128], F32, kind="ExternalInput").ap()
    o = nc.dram_tensor("o", [n_heads, 128, NT * DH], F32, kind="ExternalOutput").ap()

    with tile.TileContext(nc) as tc:
        _body(nc, tc, qp, kp, vx, cosf, sinf, tri, ident, o,
              n_heads=n_heads, S=S, chunk=chunk)

    nc.compile()
    return nc


def _body(nc, tc, qp, kp, vx, cosf, sinf, tri, ident, o, *, n_heads, S, chunk):
    from contextlib import ExitStack

    assert chunk == 512
    NT = S // 128
    npairs = n_heads // 2
    nchunks = S // chunk

    with ExitStack() as ctx:
        cpool = ctx.enter_context(tc.tile_pool(name="const", bufs=1))
        prep = ctx.enter_context(tc.tile_pool(name="prep", bufs=2))
        qkt = ctx.enter_context(tc.tile_pool(name="qkt", bufs=1))
        expp = ctx.enter_context(tc.tile_pool(name="expp", bufs=3))
        normp = ctx.enter_context(tc.tile_pool(name="normp", bufs=4))
        outp = ctx.enter_context(tc.tile_pool(name="outp", bufs=3))
        obuf = ctx.enter_context(tc.tile_pool(name="obuf", bufs=4))
        ps_s = ctx.enter_context(tc.tile_pool(name="ps_s", bufs=2, space="PSUM"))
        ps_oa = ctx.enter_context(tc.tile_pool(name="ps_oa", bufs=1, space="PSUM"))
        ps_ob = ctx.enter_context(tc.tile_pool(name="ps_ob", bufs=1, space="PSUM"))
        ps_t = ctx.enter_context(tc.tile_pool(name="ps_t", bufs=2, space="PSUM"))

        # ---- first critical load: pair-0 K natural tile, ahead of the
        # constant tables on the sync ring (it heads the RoPE chain)
        natK0 = prep.tile([128, NT * 128], F32, tag="nat", name="natK0")
        half = NT * 128 // 2
        nc.sync.dma_start(natK0[:, 0:half], kp[0][:, 0:half])

        # ---- constants
        cos_t = cpool.tile([128, NT * 128], BF16, tag="cos")
        nc.sync.dma_start(cos_t[:], cosf[:])
        sin_t = cpool.tile([128, NT * 128], BF16, tag="sin")
        nc.sync.dma_start(sin_t[:], sinf[:])
        tri_t = cpool.tile([128, 256], BF16, tag="tri")
        nc.sync.dma_start(tri_t[:], tri[:])
        id_t = cpool.tile([128, 128], F32, tag="id")
        nc.sync.dma_start(id_t[:], ident[:])
        nc.sync.dma_start(natK0[:, half:], kp[0][:, half:])
        vts = []
        for h in range(n_heads):
            vt = cpool.tile([128, NT * 65], BF16, tag=f"v{h}", name=f"v{h}")
            nc.sync.dma_start(vt[:], vx[h])
            vts.append(vt)

        # ---- RoPE + transpose prep: build Q^T / K^T (two heads stacked).
        # K chains first; pair-0 XBAR transposes alternate between the two
        # HWDGE issuing engines (sync / scalar) for latency, pair-1 goes to
        # sync only (scalar is busy with the exp stream by then).
        qT = [qkt.tile([128, NT * 128], BF16, tag=f"qT{p}", name=f"qT{p}")
              for p in range(npairs)]
        kT = [qkt.tile([128, NT * 128], BF16, tag=f"kT{p}", name=f"kT{p}")
              for p in range(npairs)]

        chains = []
        for pr in range(npairs):
            chains.append((kp[pr], kT[pr]))
            chains.append((qp[pr], qT[pr]))
        for ci, (src_ap, dstT) in enumerate(chains):
            if ci == 0:
                nat = natK0
            else:
                nat = prep.tile([128, NT * 128], F32, tag="nat")
                nc.gpsimd.dma_start(nat[:], src_ap)
            n3 = nat[:].rearrange("p (t d) -> p t d", d=128)
            s3 = sin_t[:].rearrange("p (t d) -> p t d", d=128)
            c3 = cos_t[:].rearrange("p (t d) -> p t d", d=128)
            t1 = prep.tile([128, NT * 128], F32, tag="t1")
            t2 = prep.tile([128, NT * 128], F32, tag="t2")
            t13 = t1[:].rearrange("p (t d) -> p t d", d=128)
            t23 = t2[:].rearrange("p (t d) -> p t d", d=128)
            ro = prep.tile([128, NT * 128], BF16, tag="ro")
            r3 = ro[:].rearrange("p (t d) -> p t d", d=128)
            # t1 = x * cos ; t2_even = x_odd * (-sin)_even ;
            # t2_odd = x_even * sin_odd ; out = t1 + t2   (bf16)
            nc.vector.tensor_mul(t1[:], nat[:], cos_t[:])
            nc.vector.tensor_mul(t23[:, :, 0::2], n3[:, :, 1::2], s3[:, :, 0::2])
            nc.vector.tensor_mul(t23[:, :, 1::2], n3[:, :, 0::2], s3[:, :, 1::2])
            nc.vector.tensor_add(ro[:], t1[:], t2[:])
            for t in range(NT):
                eng = (nc.sync if t % 2 == 0 else nc.scalar) if ci < 2 else nc.sync
                eng.dma_start_transpose(
                    dstT[:, t * 128:(t + 1) * 128], r3[:, t, :]
                )

        # ---- PE warm-up: dummy matmuls against kT[0] (ready before qT[0])
        # so the HAM clock gate reaches 8/8 before the first real matmul.
        s_d = ps_s.tile([128, 1024], F32, tag="s")
        for i in range(24):
            nc.tensor.matmul(
                s_d[:, 0:512], kT[0][0:64, 0:128], kT[0][0:64, 0:512],
                start=True, stop=True,
            )

        # ---- scores / softmax / attn@V: head pairs, 512-wide q-chunks.
        # Head A (partitions 0:64) and head B (64:128) issue adjacent
        # matmul1s into different PSUM banks -> concurrent row-group
        # execution in the PE array.  One ACTIVATE covers both heads'
        # scores via a strided (128, 2, 512-rel) access pattern.
        pending_norm = []

        def flush_norm():
            while pending_norm:
                pending_norm.pop(0)()

        for pr in range(npairs):
            hA, hB = 2 * pr, 2 * pr + 1
            qA, kA = qT[pr][0:64, :], kT[pr][0:64, :]
            qB, kB = qT[pr][64:128, :], kT[pr][64:128, :]
            v3A = vts[hA][:].rearrange("p (t j) -> p t j", j=65)
            v3B = vts[hB][:].rearrange("p (t j) -> p t j", j=65)
            obA = obuf.tile([128, NT * DH], F32, tag="ob", name="obA")
            obB = obuf.tile([128, NT * DH], F32, tag="ob", name="obB")
            for qc in range(nchunks):
                q0 = qc * chunk
                kpc = chunk // 128
                outA = ps_oa.tile([65, 512], F32, tag="outa")
                outB = ps_ob.tile([65, 512], F32, tag="outb")
                ktmax = (qc + 1) * kpc

                stage = []  # 1-round-deferred exp/mask/mm2
                for kt in range(ktmax):
                    rel = max(128 * kt, q0) - q0
                    s_t = ps_s.tile([128, 1024], F32, tag="s")
                    nc.tensor.matmul(
                        s_t[:, rel:512],
                        kA[:, kt * 128:(kt + 1) * 128],
                        qA[:, q0 + rel:q0 + 512],
                        start=True, stop=True,
                    )
                    nc.tensor.matmul(
                        s_t[:, 512 + rel:1024],
                        kB[:, kt * 128:(kt + 1) * 128],
                        qB[:, q0 + rel:q0 + 512],
                        start=True, stop=True,
                    )

                    def consume(kt=kt, rel=rel, s_t=s_t, ktmax=ktmax, qc=qc):
                        diag = kt >= qc * kpc
                        last = kt == ktmax - 1
                        s3v = s_t[:].rearrange("p (x q) -> p x q", x=2)
                        ex = expp.tile([128, 1024], BF16, tag="ex")
                        e3 = ex[:].rearrange("p (x q) -> p x q", x=2)
                        nc.scalar.activation(
                            e3[:, :, rel:], s3v[:, :, rel:], EXP, scale=0.125
                        )
                        if diag:
                            # causal mask on the diagonal 128-col block
                            nc.vector.tensor_mul(
                                e3[:, :, rel:rel + 128],
                                e3[:, :, rel:rel + 128],
                                tri_t[:].rearrange("p (x q) -> p x q", x=2),
                            )
                        for half, (out_t, v3) in enumerate(
                            ((outA, v3A), (outB, v3B))
                        ):
                            nc.tensor.matmul(
                                out_t[:, rel:512],
                                v3[:, kt, :],
                                ex[:, 512 * half + rel:512 * half + 512],
                                start=(kt == 0), stop=last,
                            )

                    stage.append(consume)
                    if len(stage) > 1:
                        stage.pop(0)()
                while stage:
                    stage.pop(0)()

                # copy the accumulated chunks out of PSUM promptly
                soA = normp.tile([65, 512], F32, tag="so", name="soA")
                nc.vector.tensor_copy(soA[:], outA[:])
                soB = normp.tile([65, 512], F32, tag="so", name="soB")
                nc.vector.tensor_copy(soB[:], outB[:])

                def norm(qc=qc, soA=soA, soB=soB, obA=obA, obB=obB,
                         hA=hA, hB=hB):
                    for so, ob, hh in ((soA, obA, hA), (soB, obB, hB)):
                        for j in range(chunk // 128):
                            tr = ps_t.tile([128, 65], F32, tag="tr")
                            nc.tensor.transpose(
                                tr[:], so[:, j * 128:(j + 1) * 128],
                                id_t[0:65, 0:65]
                            )
                            rc = outp.tile([128, 1], F32, tag="rc")
                            nc.vector.reciprocal(rc[:], tr[:, 64:65])
                            jj = qc * (chunk // 128) + j
                            nc.vector.tensor_scalar_mul(
                                ob[:, jj * DH:(jj + 1) * DH], tr[:, 0:DH], rc[:]
                            )
                        c0 = qc * (chunk // 128) * DH
                        c1 = c0 + (chunk // 128) * DH
                        nc.sync.dma_start(o[hh][:, c0:c1], ob[:, c0:c1])

                # run the *previous* chunk's normalization now (its deps are
                # long satisfied) so PE isn't stalled right at chunk start
                flush_norm()
                pending_norm.append(norm)
            flush_norm()
        flush_norm()


# ---------------------------------------------------------------- host side


def _rope_tables(S):
    position = np.arange(S, dtype=np.float32)[:, None]
    div = (np.float32(10000.0)
           ** (np.arange(0, DH, 2, dtype=np.float32) / np.float32(DH)))
    div = np.repeat(div.astype(np.float32), 2)
    ang = position / div[None, :]
    cos = np.cos(ang).astype(np.float32)
    sin = np.sin(ang).astype(np.float32)
    sgn = np.ones(DH, np.float32)
    sgn[0::2] = -1.0
    return cos, sin * sgn


def _fold(tab, S):
    # (S, DH) -> (128, NT, DH): [p, t, d] = tab[t*128 + p, d]
    NT = S // 128
    return np.ascontiguousarray(tab.reshape(NT, 128, DH).transpose(1, 0, 2))


def host_inputs(qh, kh, vh, S):
    """Per-core input prep.  qh/kh/vh: (n_heads, S, DH) fp32."""
    n_heads = qh.shape[0]
    NT = S // 128
    npairs = n_heads // 2

    def pack_pairs(x):
        # (n_heads, S, DH) -> (npairs, 128, NT*128), two heads interleaved
        a = x.reshape(npairs, 2, NT, 128, DH).transpose(0, 3, 2, 1, 4)
        return np.ascontiguousarray(a.reshape(npairs, 128, NT * 128))

    vt = vh.reshape(n_heads, NT, 128, DH).transpose(0, 2, 1, 3)  # (h,128,NT,DH)
    vext = np.concatenate(
        [vt, np.ones((n_heads, 128, NT, 1), np.float32)], axis=3
    ).astype(ml_dtypes.bfloat16)

    cos, sinS = _rope_tables(S)
    cosf = _fold(cos, S)
    sinf = _fold(sinS, S)
    # duplicate along d for the two stacked heads -> (128, NT, 128)
    cosf2 = np.concatenate([cosf, cosf], axis=2)
    sinf2 = np.concatenate([sinf, sinf], axis=2)

    tri1 = np.triu(np.ones((128, 128), np.float32))
    tri = np.concatenate([tri1, tri1], axis=1).astype(ml_dtypes.bfloat16)
    ident = np.eye(128, dtype=np.float32)

    return {
        "qp": pack_pairs(qh),
        "kp": pack_pairs(kh),
        "vx": np.ascontiguousarray(vext.reshape(n_heads, 128, NT * 65)),
        "cosf": np.ascontiguousarray(
            cosf2.reshape(128, NT * 128)).astype(ml_dtypes.bfloat16),
        "sinf": np.ascontiguousarray(
            sinf2.reshape(128, NT * 128)).astype(ml_dtypes.bfloat16),
        "tri": tri,
        "ident": ident,
    }


_NC_CACHE = {}


def _get_nc():
    if "nc" not in _NC_CACHE:
        _NC_CACHE["nc"] = build_nc()
    return _NC_CACHE["nc"]


def kernel(q, k, v):
    q = np.asarray(q)
    k = np.asarray(k)
    v = np.asarray(v)
    nc = _get_nc()

    # faithful raw-view head split (matches torch .view semantics)
    qh = q.reshape(B * H, S_FULL, DH)
    kh = k.reshape(B * H, S_FULL, DH)
    vh = v.reshape(B * H, S_FULL, DH)

    in_maps = []
    for c in range(N_CORES):
        sl = slice(c * HEADS_PER_CORE, (c + 1) * HEADS_PER_CORE)
        in_maps.append(host_inputs(qh[sl], kh[sl], vh[sl], S_FULL))

    res = run_bass_kernel_spmd(nc, in_maps, list(range(N_CORES)))

    NT = S_FULL // 128
    out = np.empty((B * H, S_FULL, DH), np.float32)
    for c in range(N_CORES):
        oc = res.results[c]["o"]  # (heads, 128, NT*DH)
        oc = oc.reshape(HEADS_PER_CORE, 128, NT, DH).transpose(0, 2, 1, 3)
        out[c * HEADS_PER_CORE:(c + 1) * HEADS_PER_CORE] = oc.reshape(
            HEADS_PER_CORE, S_FULL, DH
        )
    return out.reshape(B, S_FULL, H * DH)



# revision 5
# speedup vs baseline: 1.2071x; 1.2071x over previous
"""Multi-head self-attention (RoPE + causal softmax) Bass kernel for TRN2.

Problem: B=2, H=16, S=2048, D_HEAD=64, fp32 I/O.
Sharding: 32 head-instances (B*H) split 4-per-core across 8 NeuronCores;
no cross-device communication.

v2 design (per core, 4 heads = 2 stacked pairs):
  - Q,K ship host-pre-transposed as bf16 pair tiles (128 partitions =
    [headA d0..63 | headB d0..63], free = s), plus pair-swapped sign-folded
    copies (qs/ks).  RoPE on DVE is then 3 all-bf16 2x-mode tensor ops per
    chain: rot = q*cosT + qshuf*sinT.  No XBAR DMA transposes at all.
  - Scores computed transposed per 128-row k-tile into [128, 2x512] PSUM
    (head A cols 0:512, head B 512:1024), causally trimmed.
  - exp(s/8) per k-tile, engine-split: ScalarE exact-exp by default; large
    context (non-diagonal) tiles can be routed to DVE / GPSIMD using a
    bf16 Schraudolph bit-trick (i16 = round(s*A + B) reinterpreted bf16).
  - Diagonal blocks masked by a 128x256 triangular multiply on DVE.
  - attn@[V|1] accumulates out^T(65 x 1024, A|B halves) over k-tiles;
    row 64 is the softmax denominator.
  - Per chunk: one DVE copy PSUM->SBUF, 65x128 PE transposes, per-partition
    reciprocal + tensor_scalar_mul -> bf16 output tiles, DMA'd to DRAM.
  - Continuous PE warmup bridges the DMA/RoPE prep so HAM reaches 8/8
    before the first real matmul and never re-throttles.
"""

import math

import numpy as np
import ml_dtypes

import concourse.bass as bass
import concourse.tile as tile
from concourse import bacc, mybir
from concourse.bass_utils import run_bass_kernel_spmd

F32 = mybir.dt.float32
BF16 = mybir.dt.bfloat16
I16 = mybir.dt.int16
EXP = mybir.ActivationFunctionType.Exp
MULT = mybir.AluOpType.mult
ADD = mybir.AluOpType.add

B, H, S_FULL, DH = 2, 16, 2048, 64
N_CORES = 8
HEADS_PER_CORE = (B * H) // N_CORES  # 4

# Schraudolph fast-exp constants for bf16 (exp(x) with x = s * 0.125):
# i16 = round(s * FE_A + FE_B); bits(i16) viewed as bf16 ~= exp(s/8).
# C = 7.38 zeroes the mean multiplicative bias (max rel err ~4%).
FE_A = 128.0 / math.log(2.0) * 0.125
FE_B = 127.0 * 128.0 - 7.38

# exp engine split: number of non-diagonal k-tiles (per chunk, from the
# oldest kt up) that go to GPSIMD / DVE instead of ScalarE.  Diagonal
# tiles and chunk qc=0 always use exact ScalarE exp.
EXP_SPLIT = {"gps": 0, "dve": 0}
WARMUP_MMS = 30


# ---------------------------------------------------------------- device IR


def build_nc(n_heads=HEADS_PER_CORE, S=S_FULL, chunk=512, num_devices=N_CORES,
             exp_split=None):
    """Build + compile the per-core Bass program (same program on all cores)."""
    NT = S // 128
    npairs = n_heads // 2

    nc = bacc.Bacc(
        "TRN2", target_bir_lowering=False, debug=False, num_devices=num_devices
    )

    qt = nc.dram_tensor("qt", [npairs, 128, S], BF16, kind="ExternalInput").ap()
    qs = nc.dram_tensor("qs", [npairs, 128, S], BF16, kind="ExternalInput").ap()
    kt = nc.dram_tensor("kt", [npairs, 128, S], BF16, kind="ExternalInput").ap()
    ks = nc.dram_tensor("ks", [npairs, 128, S], BF16, kind="ExternalInput").ap()
    vx = nc.dram_tensor("vx", [n_heads, 128, NT * 65], BF16, kind="ExternalInput").ap()
    cosf = nc.dram_tensor("cosf", [128, S], BF16, kind="ExternalInput").ap()
    sinf = nc.dram_tensor("sinf", [128, S], BF16, kind="ExternalInput").ap()
    tri = nc.dram_tensor("tri", [128, 256], BF16, kind="ExternalInput").ap()
    ident = nc.dram_tensor("ident", [65, 65], F32, kind="ExternalInput").ap()
    o = nc.dram_tensor("o", [n_heads, 128, NT * DH], BF16, kind="ExternalOutput").ap()

    with tile.TileContext(nc) as tc:
        _body(nc, tc, qt, qs, kt, ks, vx, cosf, sinf, tri, ident, o,
              n_heads=n_heads, S=S, chunk=chunk,
              exp_split=exp_split if exp_split is not None else EXP_SPLIT)

    nc.compile()
    return nc


def _body(nc, tc, qt, qs, kt_, ks, vx, cosf, sinf, tri, ident, o, *,
          n_heads, S, chunk, exp_split):
    from contextlib import ExitStack

    assert chunk == 512
    NT = S // 128
    npairs = n_heads // 2
    nchunks = S // chunk
    kpc = chunk // 128
    half = S // 2

    with ExitStack() as ctx:
        cpool = ctx.enter_context(tc.tile_pool(name="const", bufs=1))
        raw = ctx.enter_context(tc.tile_pool(name="raw", bufs=1))
        rot = ctx.enter_context(tc.tile_pool(name="rot", bufs=1))
        prep = ctx.enter_context(tc.tile_pool(name="prep", bufs=2))
        expp = ctx.enter_context(tc.tile_pool(name="expp", bufs=3))
        sop = ctx.enter_context(tc.tile_pool(name="sop", bufs=2))
        rcp = ctx.enter_context(tc.tile_pool(name="rcp", bufs=4))
        obuf = ctx.enter_context(tc.tile_pool(name="obuf", bufs=1))
        ps_s = ctx.enter_context(tc.tile_pool(name="ps_s", bufs=2, space="PSUM"))
        ps_o = ctx.enter_context(tc.tile_pool(name="ps_o", bufs=1, space="PSUM"))
        ps_t = ctx.enter_context(tc.tile_pool(name="ps_t", bufs=2, space="PSUM"))

        # ---- warmup seed + ACT exp-table preload (both ready at t~0)
        wt = cpool.tile([128, 512], BF16, tag="wt")
        nc.vector.memset(wt[:], 0.25)
        dme = cpool.tile([128, 8], BF16, tag="dme")
        nc.scalar.activation(dme[:], wt[:, 0:8], EXP, scale=0.125)

        # ---- constants + inputs.  sync queue carries the critical pair-0
        # chain; gpsimd (SWDGE) carries V and pair-1.
        cos_t = cpool.tile([128, S], BF16, tag="cos")
        sin_t = cpool.tile([128, S], BF16, tag="sin")
        nc.sync.dma_start(cos_t[:, 0:half], cosf[:, 0:half])
        nc.sync.dma_start(sin_t[:, 0:half], sinf[:, 0:half])

        kraw = [raw.tile([128, S], BF16, tag=f"kr{p}", name=f"kr{p}") for p in range(npairs)]
        ksraw = [raw.tile([128, S], BF16, tag=f"ks{p}", name=f"ksr{p}") for p in range(npairs)]
        qraw = [raw.tile([128, S], BF16, tag=f"qr{p}", name=f"qr{p}") for p in range(npairs)]
        qsraw = [raw.tile([128, S], BF16, tag=f"qs{p}", name=f"qsr{p}") for p in range(npairs)]
        nc.sync.dma_start(kraw[0][:, 0:half], kt_[0][:, 0:half])
        nc.sync.dma_start(ksraw[0][:, 0:half], ks[0][:, 0:half])
        nc.sync.dma_start(qraw[0][:, 0:half], qt[0][:, 0:half])
        nc.sync.dma_start(qsraw[0][:, 0:half], qs[0][:, 0:half])

        tri_t = cpool.tile([128, 256], BF16, tag="tri")
        nc.sync.dma_start(tri_t[:], tri[:])
        id_t = cpool.tile([65, 65], F32, tag="id")
        nc.sync.dma_start(id_t[:], ident[:])

        nc.sync.dma_start(cos_t[:, half:], cosf[:, half:])
        nc.sync.dma_start(sin_t[:, half:], sinf[:, half:])
        nc.sync.dma_start(kraw[0][:, half:], kt_[0][:, half:])
        nc.sync.dma_start(ksraw[0][:, half:], ks[0][:, half:])
        nc.sync.dma_start(qraw[0][:, half:], qt[0][:, half:])
        nc.sync.dma_start(qsraw[0][:, half:], qs[0][:, half:])

        vts = []
        for h in range(n_heads):
            vt = cpool.tile([128, NT * 65], BF16, tag=f"v{h}", name=f"v{h}")
            nc.gpsimd.dma_start(vt[:], vx[h])
            vts.append(vt)
        for p in range(1, npairs):
            nc.gpsimd.dma_start(kraw[p][:], kt_[p])
            nc.gpsimd.dma_start(ksraw[p][:], ks[p])
            nc.gpsimd.dma_start(qraw[p][:], qt[p])
            nc.gpsimd.dma_start(qsraw[p][:], qs[p])

        # ---- PE warmup: bridge the prep phase with back-to-back matmuls so
        # the HAM clock-gate reaches 8/8 before (and stays through) the
        # first real matmul.
        s_d = ps_s.tile([128, 1024], F32, tag="s")
        for _ in range(WARMUP_MMS):
            nc.tensor.matmul(s_d[:, 0:512], wt[0:64, 0:128], wt[0:64, 0:512],
                             start=True, stop=True)

        # ---- RoPE: rot = rawT (.) cosT + shufT (.) sinT, all bf16, two
        # halves per chain for pipelining.
        kT = [rot.tile([128, S], BF16, tag=f"kT{p}", name=f"kT{p}") for p in range(npairs)]
        qT = [rot.tile([128, S], BF16, tag=f"qT{p}", name=f"qT{p}") for p in range(npairs)]
        chains = []
        for pr in range(npairs):
            chains.append((kraw[pr], ksraw[pr], kT[pr]))
            chains.append((qraw[pr], qsraw[pr], qT[pr]))
        for nat, shuf, dst in chains:
            for h0, h1 in ((0, half), (half, S)):
                t1 = prep.tile([128, half], BF16, tag="t1")
                t2 = prep.tile([128, half], BF16, tag="t2")
                nc.vector.tensor_mul(t1[:], nat[:, h0:h1], cos_t[:, h0:h1])
                nc.vector.tensor_mul(t2[:], shuf[:, h0:h1], sin_t[:, h0:h1])
                nc.vector.tensor_add(dst[:, h0:h1], t1[:], t2[:])

        # ---- per-head output buffers (bf16), DMA'd out per chunk
        obs = [obuf.tile([128, NT * DH], BF16, tag=f"ob{h}", name=f"ob{h}")
               for h in range(n_heads)]

        # ---- scores / softmax / attn@V
        pending_norm = []

        def flush_norm():
            while pending_norm:
                pending_norm.pop(0)()

        for pr in range(npairs):
            hA, hB = 2 * pr, 2 * pr + 1
            qA, kA = qT[pr][0:64, :], kT[pr][0:64, :]
            qB, kB = qT[pr][64:128, :], kT[pr][64:128, :]
            v3A = vts[hA][:].rearrange("p (t j) -> p t j", j=65)
            v3B = vts[hB][:].rearrange("p (t j) -> p t j", j=65)
            for qc in range(nchunks):
                q0 = qc * chunk
                ktmax = (qc + 1) * kpc
                out_t = ps_o.tile([65, 1024], F32, tag="out")

                # engine assignment for this chunk's non-diag tiles
                ndiag = qc * kpc  # tiles kt < ndiag are full (non-diagonal)
                eng_for = {}
                n_g = min(exp_split["gps"], ndiag)
                n_d = min(exp_split["dve"], ndiag - n_g)
                for i in range(ndiag):
                    if i < n_g:
                        eng_for[i] = "gps"
                    elif i < n_g + n_d:
                        eng_for[i] = "dve"
                    else:
                        eng_for[i] = "act"

                stage = []  # 1-round-deferred exp/mask/mm2
                for kt2 in range(ktmax):
                    rel = max(128 * kt2, q0) - q0
                    s_t = ps_s.tile([128, 1024], F32, tag="s")
                    nc.tensor.matmul(
                        s_t[:, rel:512],
                        kA[:, kt2 * 128:(kt2 + 1) * 128],
                        qA[:, q0 + rel:q0 + 512],
                        start=True, stop=True,
                    )
                    nc.tensor.matmul(
                        s_t[:, 512 + rel:1024],
                        kB[:, kt2 * 128:(kt2 + 1) * 128],
                        qB[:, q0 + rel:q0 + 512],
                        start=True, stop=True,
                    )

                    def consume(kt2=kt2, rel=rel, s_t=s_t, ktmax=ktmax, qc=qc,
                                eng=eng_for.get(kt2, "act")):
                        diag = kt2 >= qc * kpc
                        last = kt2 == ktmax - 1
                        s3v = s_t[:].rearrange("p (x q) -> p x q", x=2)
                        ex = expp.tile([128, 1024], BF16, tag="ex")
                        e3 = ex[:].rearrange("p (x q) -> p x q", x=2)
                        if eng == "act" or diag:
                            nc.scalar.activation(
                                e3[:, :, rel:], s3v[:, :, rel:], EXP, scale=0.125
                            )
                        else:
                            e3i = ex[:].bitcast(I16).rearrange(
                                "p (x q) -> p x q", x=2)
                            engine = nc.gpsimd if eng == "gps" else nc.vector
                            engine.tensor_scalar(
                                e3i[:, :, rel:], s3v[:, :, rel:],
                                FE_A, FE_B, MULT, ADD,
                            )
                        if diag:
                            nc.vector.tensor_mul(
                                e3[:, :, rel:rel + 128],
                                e3[:, :, rel:rel + 128],
                                tri_t[:].rearrange("p (x q) -> p x q", x=2),
                            )
                        for hf, v3 in ((0, v3A), (1, v3B)):
                            nc.tensor.matmul(
                                out_t[:, 512 * hf + rel:512 * hf + 512],
                                v3[:, kt2, :],
                                ex[:, 512 * hf + rel:512 * hf + 512],
                                start=(kt2 == 0), stop=last,
                            )

                    stage.append(consume)
                    if len(stage) > 1:
                        stage.pop(0)()
                while stage:
                    stage.pop(0)()

                # drain the PSUM accumulators promptly (frees banks for the
                # next chunk's first mm2)
                so = sop.tile([65, 1024], F32, tag="so")
                nc.vector.tensor_copy(so[:], out_t[:])

                def norm(qc=qc, so=so, hA=hA, hB=hB):
                    for hf, hh in ((0, hA), (1, hB)):
                        ob = obs[hh]
                        for j in range(kpc):
                            tr = ps_t.tile([128, 65], F32, tag="tr")
                            nc.tensor.transpose(
                                tr[:], so[:, hf * 512 + j * 128:
                                          hf * 512 + (j + 1) * 128],
                                id_t[:],
                            )
                            rc = rcp.tile([128, 1], F32, tag="rc")
                            nc.vector.reciprocal(rc[:], tr[:, 64:65])
                            jj = qc * kpc + j
                            nc.vector.tensor_scalar_mul(
                                ob[:, jj * DH:(jj + 1) * DH], tr[:, 0:DH], rc[:]
                            )
                        c0 = qc * kpc * DH
                        nc.sync.dma_start(
                            o[hh][:, c0:c0 + kpc * DH], ob[:, c0:c0 + kpc * DH]
                        )

                # run the previous chunk's normalization now (deps long
                # satisfied) so PE isn't stalled right at chunk start
                flush_norm()
                pending_norm.append(norm)
            flush_norm()
        flush_norm()


# ---------------------------------------------------------------- host side


def _rope_tables_T(S):
    """Transposed tables: cosT/sinT[p, s] for the stacked pair layout
    (partitions = [headA d0..63 | headB d0..63])."""
    d = np.arange(DH, dtype=np.float32)
    div = np.float32(10000.0) ** ((d // 2 * 2).astype(np.float32) / np.float32(DH))
    pos = np.arange(S, dtype=np.float32)
    ang = pos[None, :] / div[:, None]          # (64, S)
    cosT = np.cos(ang)
    sinT = np.sin(ang)
    cosT = np.concatenate([cosT, cosT], axis=0)  # (128, S)
    sinT = np.concatenate([sinT, sinT], axis=0)
    return cosT.astype(ml_dtypes.bfloat16), sinT.astype(ml_dtypes.bfloat16)


def host_inputs(qh, kh, vh, S):
    """Per-core input prep.  qh/kh/vh: (n_heads, S, DH) fp32."""
    n_heads = qh.shape[0]
    NT = S // 128
    npairs = n_heads // 2

    def pack_T(x):
        # (n_heads, S, DH) -> (npairs, 128, S): [pr, 64*a + d, s]
        a = x.reshape(npairs, 2, S, DH).transpose(0, 1, 3, 2)  # (pr, 2, DH, S)
        return np.ascontiguousarray(a.reshape(npairs, 128, S))

    def shuffle_sign(xT):
        # row 2j <- -x[2j+1], row 2j+1 <- +x[2j]  (within each 64-row head)
        out = np.empty_like(xT)
        out[:, 0::2] = -xT[:, 1::2]
        out[:, 1::2] = xT[:, 0::2]
        return out

    qT = pack_T(qh)
    kT = pack_T(kh)
    qS = shuffle_sign(qT)
    kS = shuffle_sign(kT)

    vt = vh.reshape(n_heads, NT, 128, DH).transpose(0, 2, 1, 3)  # (h,128,NT,DH)
    vext = np.concatenate(
        [vt, np.ones((n_heads, 128, NT, 1), np.float32)], axis=3
    ).astype(ml_dtypes.bfloat16)

    cosT, sinT = _rope_tables_T(S)

    tri1 = np.triu(np.ones((128, 128), np.float32))
    tri = np.concatenate([tri1, tri1], axis=1).astype(ml_dtypes.bfloat16)
    ident = np.eye(65, dtype=np.float32)

    bf = ml_dtypes.bfloat16
    return {
        "qt": qT.astype(bf),
        "qs": qS.astype(bf),
        "kt": kT.astype(bf),
        "ks": kS.astype(bf),
        "vx": np.ascontiguousarray(vext.reshape(n_heads, 128, NT * 65)),
        "cosf": np.ascontiguousarray(cosT),
        "sinf": np.ascontiguousarray(sinT),
        "tri": tri,
        "ident": ident,
    }


_NC_CACHE = {}


def _get_nc():
    if "nc" not in _NC_CACHE:
        _NC_CACHE["nc"] = build_nc()
    return _NC_CACHE["nc"]


def kernel(q, k, v):
    q = np.asarray(q)
    k = np.asarray(k)
    v = np.asarray(v)
    nc = _get_nc()

    # faithful raw-view head split (matches torch .view semantics)
    qh = q.reshape(B * H, S_FULL, DH)
    kh = k.reshape(B * H, S_FULL, DH)
    vh = v.reshape(B * H, S_FULL, DH)

    in_maps = []
    for c in range(N_CORES):
        sl = slice(c * HEADS_PER_CORE, (c + 1) * HEADS_PER_CORE)
        in_maps.append(host_inputs(qh[sl], kh[sl], vh[sl], S_FULL))

    res = run_bass_kernel_spmd(nc, in_maps, list(range(N_CORES)))

    NT = S_FULL // 128
    out = np.empty((B * H, S_FULL, DH), np.float32)
    for c in range(N_CORES):
        oc = np.asarray(res.results[c]["o"]).astype(np.float32)
        oc = oc.reshape(HEADS_PER_CORE, 128, NT, DH).transpose(0, 2, 1, 3)
        out[c * HEADS_PER_CORE:(c + 1) * HEADS_PER_CORE] = oc.reshape(
            HEADS_PER_CORE, S_FULL, DH
        )
    return out.reshape(B, S_FULL, H * DH)
